# revision 8
# baseline (speedup 1.0000x reference)
"""Trainium2 Bass kernel for the nn_CA depthwise-conv CA step.

Pipeline per image: depthwise 5x5 conv (D4-symmetrized, zero-mean kernel,
SAME padding) + bias + leaky_relu; 1x1 conv (8x8 channel GEMM) + bias +
leaky_relu; 1x1 conv + bias + psi residual + tanh.

Strategy: pure data parallel over 8 NeuronCores (256 images each).
On-chip layout: partition p = h*16 + ylocal, one tile per y-quarter
(y = qy*16 + ylocal) holding ALL 8 channels; free dim = (image, x).
All data is fp16.

With all 8 channels in the partition dim, each 8x8 channel GEMM is a
SINGLE matmul per quarter (lhsT = W[co,ci] (x) I16) — stages 2+3 are 8
matmuls per 8-image step vs 32 in a (2ch x 64y) pair layout. The
depthwise conv uses the D4 x-symmetry (kernel columns 0==4, 1==3): per
quarter it is 3 banded-Toeplitz matmuls (center column, s1 =
psi(x-1)+psi(x+1), s2 = psi(x-2)+psi(x+2); s tiles built on the idle
vector engine), plus corner-block "spill" matmuls carrying the
y-convolution across quarter boundaries (up to 6 per quarter). 38
matmuls per step total. Per-quarter PSUM tiles keep dependency tracking
fine-grained: conv accumulators use 2 banks double-buffered, leaving 6
banks for the GEMM/act/residual chain. The psi residual is added by the
vector engine in PSUM; biases + activations are fused into ScalarE
activation instructions reading PSUM.
"""

import numpy as np

F16 = np.dtype(np.float16)

BS, H, RES = 2048, 8, 64
NCORES = 8
IPC = BS // NCORES  # images per core
SG = 32             # images per super-group (DMA granularity)
NSG = IPC // SG
S8 = 8              # images per PSUM step (512 free columns per quarter)
NST = SG // S8
XP = RES + 4        # x-padded width
YQ = 16             # y rows per quarter tile

NWM = 12            # 3 main conv + 3 spill-up + 3 spill-down + gemm2 + gemm3 + I

_CACHE = {}


def _totalistic(x):
    # D4-symmetrize 5x5 kernels over spatial dims, then remove spatial mean
    z = 0.125 * (x + x[:, :, ::-1, :] + x[:, :, :, ::-1] + x[:, :, ::-1, ::-1])
    xt = np.swapaxes(x, 2, 3)
    z = z + 0.125 * (xt + xt[:, :, ::-1, :] + xt[:, :, :, ::-1] + xt[:, :, ::-1, ::-1])
    return z - z.mean(axis=(2, 3), keepdims=True)


def _build_program(reps=1, resid_on_pe=False):
    import concourse.bacc as bacc
    import concourse.tile as tile
    from concourse import mybir

    dt = mybir.dt
    nc = bacc.Bacc("TRN2", target_bir_lowering=False, debug=False, num_devices=NCORES)
    psi = nc.dram_tensor("psi", [4, 128, IPC * XP], dt.float16, kind="ExternalInput").ap()
    wm = nc.dram_tensor("wm", [NWM, 128, 128], dt.float16, kind="ExternalInput").ap()
    bv = nc.dram_tensor("bv", [128, 2], dt.float32, kind="ExternalInput").ap()
    out = nc.dram_tensor("out", [4, 128, IPC * RES], dt.float16, kind="ExternalOutput").ap()

    LR = mybir.ActivationFunctionType.Lrelu
    TH = mybir.ActivationFunctionType.Tanh

    with tile.TileContext(nc) as tc:
        from contextlib import ExitStack

        with ExitStack() as ctx:
            const = ctx.enter_context(tc.tile_pool(name="const", bufs=1))
            psip = ctx.enter_context(tc.tile_pool(name="psip", bufs=2))
            spool = ctx.enter_context(tc.tile_pool(name="spool", bufs=2))
            opool = ctx.enter_context(tc.tile_pool(name="opool", bufs=2))
            zpool = ctx.enter_context(tc.tile_pool(name="zpool", bufs=2))
            psum = ctx.enter_context(tc.tile_pool(name="psum", bufs=5, space="PSUM"))

            wt = const.tile([128, NWM * 128], dt.float16)
            nc.sync.dma_start(
                wt[:].rearrange("p (m k) -> p m k", m=NWM),
                wm.rearrange("m p k -> p m k"),
            )
            bt = const.tile([128, 2], dt.float32)
            nc.sync.dma_start(bt[:], bv[:])

            def W(i):
                return wt[:, i * 128 : (i + 1) * 128]

            for rep in range(reps):
              for sg in range(NSG):
                ptiles = []
                for v in range(4):
                    t = psip.tile([128, SG * XP], dt.float16, tag=f"psi{v}", name=f"psi_t{v}_{sg}")
                    nc.sync.dma_start(
                        t[:], psi[v, :, sg * SG * XP : (sg + 1) * SG * XP]
                    )
                    ptiles.append(t)
                otiles = [
                    opool.tile([128, SG * RES], dt.float16, tag=f"o{v}", name=f"o_t{v}_{sg}")
                    for v in range(4)
                ]

                svcache = {}

                def views(v, st):
                    # (center, s1, s2) rhs views of quarter v for step st;
                    # s1/s2 shifted sums built per-step on the vector engine
                    pt3 = ptiles[v][:].rearrange("p (i x) -> p i x", i=SG)[
                        :, st * S8 : (st + 1) * S8, :
                    ]
                    if (v, st) not in svcache:
                        t1 = spool.tile([128, S8 * RES], dt.float16, tag=f"s1_{v}_{st % 2}", name=f"s1_{v}_{sg}_{st}")
                        t2 = spool.tile([128, S8 * RES], dt.float16, tag=f"s2_{v}_{st % 2}", name=f"s2_{v}_{sg}_{st}")
                        nc.vector.tensor_add(
                            t1[:].rearrange("p (i x) -> p i x", i=S8),
                            pt3[:, :, 1 : 1 + RES], pt3[:, :, 3 : 3 + RES])
                        nc.vector.tensor_add(
                            t2[:].rearrange("p (i x) -> p i x", i=S8),
                            pt3[:, :, 0:RES], pt3[:, :, 4 : 4 + RES])
                        svcache[(v, st)] = (t1, t2)
                    t1, t2 = svcache[(v, st)]
                    return (pt3[:, :, 2 : 2 + RES],
                            t1[:].rearrange("p (i x) -> p i x", i=S8),
                            t2[:].rearrange("p (i x) -> p i x", i=S8))

                for st in range(NST):
                    for v in range(4):
                        views(v, st)  # emit DVE s-adds ahead of the conv
                    # stage 1: depthwise conv + lrelu (bias1 is always 0).
                    z1 = []
                    for v in range(4):
                        cps = psum.tile(
                            [128, S8 * RES], dt.float32,
                            tag="cps", name=f"cps{sg}_{st}_{v}", bufs=3,
                        )
                        cps3 = cps[:].rearrange("p (i x) -> p i x", i=S8)
                        mm = []
                        own = views(v, st)
                        for c in range(3):  # lhsT c: x-class (0=s2,1=s1,2=center)
                            mm.append((W(c), own[2 - c]))
                        if v > 0:
                            below = views(v - 1, st)
                            for c in range(3):
                                mm.append((W(3 + c), below[2 - c]))
                        if v < 3:
                            above = views(v + 1, st)
                            for c in range(3):
                                mm.append((W(6 + c), above[2 - c]))
                        for j, (lhs, rhs) in enumerate(mm):
                            nc.tensor.matmul(
                                cps3, lhs, rhs,
                                start=(j == 0), stop=(j == len(mm) - 1),
                            )
                        z = zpool.tile(
                            [128, S8 * RES], dt.float16,
                            tag=f"z1_{v}", name=f"z1_{sg}_{st}_{v}",
                        )
                        nc.scalar.activation(z[:], cps[:], LR, alpha=0.01)
                        z1.append(z)
                    # stage 2: 8x8 channel GEMM + b2 + lrelu (one matmul/quarter)
                    z2 = []
                    for v in range(4):
                        gps = psum.tile(
                            [128, S8 * RES], dt.float32, tag="ps", name=f"g2ps{sg}_{st}_{v}",
                        )
                        nc.tensor.matmul(
                            gps[:], W(9), z1[v][:], start=True, stop=True,
                        )
                        z = zpool.tile([128, S8 * RES], dt.float16, tag=f"z2_{v}", name=f"z2_{sg}_{st}_{v}")
                        nc.scalar.activation(
                            z[:], gps[:], LR, bias=bt[:, 0:1], alpha=0.01
                        )
                        z2.append(z)
                    # stage 3: 8x8 channel GEMM + psi residual + b3 + tanh
                    for v in range(4):
                        gps = psum.tile(
                            [128, S8 * RES], dt.float32, tag="ps", name=f"g3ps{sg}_{st}_{v}",
                        )
                        pt3 = ptiles[v][:].rearrange("p (i x) -> p i x", i=SG)[
                            :, st * S8 : (st + 1) * S8, 2 : 2 + RES
                        ]
                        gps3 = gps[:].rearrange("p (i x) -> p i x", i=S8)
                        if resid_on_pe:
                            nc.tensor.matmul(gps3, W(11), pt3, start=True, stop=False)
                        nc.tensor.matmul(
                            gps[:], W(10), z2[v][:],
                            start=not resid_on_pe, stop=True,
                        )
                        if not resid_on_pe:
                            # psi residual on the vector engine
                            nc.vector.tensor_add(gps3, gps3, pt3)
                        nc.scalar.activation(
                            otiles[v][:, st * S8 * RES : (st + 1) * S8 * RES],
                            gps[:], TH, bias=bt[:, 1:2],
                        )
                for v in range(4):
                    nc.sync.dma_start(
                        out[v, :, sg * SG * RES : (sg + 1) * SG * RES],
                        otiles[v][:],
                    )

    nc.compile()
    return nc


def _host_pack(filter1, bias1, w2, b2, w3, b3):
    w = _totalistic(filter1.astype(np.float32))[:, 0]  # [8,5,5]
    wm = np.zeros((NWM, 128, 128), np.float32)
    # main banded conv matrices (same local structure for every quarter):
    # lhsT[ch*16+yi, ch*16+yo] = K[ch, (yi-yo)+2, c]
    for c in range(3):
        for ch in range(H):
            o = ch * YQ
            for yi in range(YQ):
                for yo in range(YQ):
                    d = yi - yo
                    if -2 <= d <= 2:
                        wm[c, o + yi, o + yo] = w[ch, d + 2, c]
            # spill up: src quarter u -> dst u+1; dy = ys - 16 - yd in {-2,-1}
            for ys, yd in ((14, 0), (15, 0), (15, 1)):
                wm[3 + c, o + ys, o + yd] = w[ch, (ys - 16 - yd) + 2, c]
            # spill down: src quarter u -> dst u-1; dy = ys + 16 - yd in {1,2}
            for ys, yd in ((0, 14), (0, 15), (1, 15)):
                wm[6 + c, o + ys, o + yd] = w[ch, (ys + 16 - yd) + 2, c]
    # channel GEMMs: lhsT[ci*16+y, co*16+y] = W[co, ci]
    i16 = np.eye(YQ, dtype=np.float32)
    for co in range(H):
        for ci in range(H):
            wm[9, ci * YQ : ci * YQ + YQ, co * YQ : co * YQ + YQ] = w2[co, ci] * i16
            wm[10, ci * YQ : ci * YQ + YQ, co * YQ : co * YQ + YQ] = w3[co, ci] * i16
    wm[11] = np.eye(128, dtype=np.float32)

    bvv = np.zeros((128, 2), np.float32)
    for ch in range(H):
        sl = slice(ch * YQ, ch * YQ + YQ)
        bvv[sl, 0] = b2[ch]
        bvv[sl, 1] = b3[ch]
    return wm.astype(F16), bvv


def _pack_psi(psi):
    """[BS,H,RES,RES] -> [NCORES, 4, 128, IPC*XP] fp16: tile axis = y-quarter,
    partition p = ch*16 + ylocal, free = (image, padded x)."""
    psip = np.zeros((BS, H, RES, XP), np.float32)
    psip[:, :, :, 2 : 2 + RES] = psi
    v = psip.reshape(NCORES, IPC, H, 4, YQ, XP)
    v = v.transpose(0, 3, 2, 4, 1, 5)  # [NCORES, qy, ch, ylocal, IPC, XP]
    return np.ascontiguousarray(v).reshape(NCORES, 4, 128, IPC * XP).astype(F16)


def _unpack_out(parts):
    """list of [4,128,IPC*RES] fp16 per core -> [BS,H,RES,RES] fp32"""
    v = np.stack([np.asarray(p).astype(np.float32) for p in parts])
    v = v.reshape(NCORES, 4, H, YQ, IPC, RES)
    v = v.transpose(0, 4, 2, 1, 3, 5)  # [NCORES, IPC, ch, qy, ylocal, RES]
    return np.ascontiguousarray(v).reshape(BS, H, RES, RES)


def kernel(psi, filter1, bias1, w2, b2, w3, b3):
    from concourse.bass_utils import run_bass_kernel_spmd

    psi = np.asarray(psi, dtype=np.float32)
    wm, bvv = _host_pack(
        np.asarray(filter1, np.float32),
        np.asarray(bias1, np.float32),
        np.asarray(w2, np.float32),
        np.asarray(b2, np.float32),
        np.asarray(w3, np.float32),
        np.asarray(b3, np.float32),
    )

    psit = _pack_psi(psi)

    if "nc" not in _CACHE:
        _CACHE["nc"] = _build_program()
    nc = _CACHE["nc"]

    in_maps = [{"psi": psit[c], "wm": wm, "bv": bvv} for c in range(NCORES)]
    res = run_bass_kernel_spmd(nc, in_maps, list(range(NCORES)))
    return _unpack_out([r["out"] for r in res.results])


# revision 9
# speedup vs baseline: 1.7953x; 1.7953x over previous
"""Trainium2 Bass kernel for the nn_CA depthwise-conv CA step.

Pipeline per image: depthwise 5x5 conv (D4-symmetrized, zero-mean kernel,
SAME padding) + bias + leaky_relu; 1x1 conv (8x8 channel GEMM) + bias +
leaky_relu; 1x1 conv + bias + psi residual + tanh.

Strategy: pure data parallel over 8 NeuronCores (256 images each).
On-chip layout: partition p = h*16 + ylocal, one tile per y-quarter
(y = qy*16 + ylocal) holding ALL 8 channels; free dim = (image, x).
All data is fp16.

With all 8 channels in the partition dim, each 8x8 channel GEMM is a
SINGLE matmul per quarter (lhsT = W[co,ci] (x) I16) — stages 2+3 are 8
matmuls per 8-image step vs 32 in a (2ch x 64y) pair layout. The
depthwise conv uses the D4 x-symmetry (kernel columns 0==4, 1==3): per
quarter it is 3 banded-Toeplitz matmuls (center column, s1 =
psi(x-1)+psi(x+1), s2 = psi(x-2)+psi(x+2); s tiles built on the idle
vector engine), plus corner-block "spill" matmuls carrying the
y-convolution across quarter boundaries (up to 6 per quarter). 38
matmuls per step total. Per-quarter PSUM tiles keep dependency tracking
fine-grained: conv accumulators use 2 banks double-buffered, leaving 6
banks for the GEMM/act/residual chain. The psi residual is added by the
vector engine in PSUM; biases + activations are fused into ScalarE
activation instructions reading PSUM.
"""

import numpy as np

F16 = np.dtype(np.float16)

BS, H, RES = 2048, 8, 64
NCORES = 8
IPC = BS // NCORES  # images per core
SG = 32             # images per super-group (DMA granularity)
NSG = IPC // SG
S8 = 8              # images per PSUM step (512 free columns per quarter)
NST = SG // S8
XP = RES + 4        # x-padded width
YQ = 16             # y rows per quarter tile

NWM = 12            # 3 main conv + 3 spill-up + 3 spill-down + gemm2 + gemm3 + I

_CACHE = {}


def _totalistic(x):
    # D4-symmetrize 5x5 kernels over spatial dims, then remove spatial mean
    z = 0.125 * (x + x[:, :, ::-1, :] + x[:, :, :, ::-1] + x[:, :, ::-1, ::-1])
    xt = np.swapaxes(x, 2, 3)
    z = z + 0.125 * (xt + xt[:, :, ::-1, :] + xt[:, :, :, ::-1] + xt[:, :, ::-1, ::-1])
    return z - z.mean(axis=(2, 3), keepdims=True)


def _build_program(reps=1, resid_on_pe=False):
    import concourse.bacc as bacc
    import concourse.tile as tile
    from concourse import mybir

    dt = mybir.dt
    nc = bacc.Bacc("TRN2", target_bir_lowering=False, debug=False, num_devices=NCORES)
    psi = nc.dram_tensor("psi", [4, 128, IPC * XP], dt.float16, kind="ExternalInput").ap()
    wm = nc.dram_tensor("wm", [NWM, 128, 128], dt.float16, kind="ExternalInput").ap()
    bv = nc.dram_tensor("bv", [128, 2], dt.float32, kind="ExternalInput").ap()
    out = nc.dram_tensor("out", [4, 128, IPC * RES], dt.float16, kind="ExternalOutput").ap()

    LR = mybir.ActivationFunctionType.Lrelu
    TH = mybir.ActivationFunctionType.Tanh

    with tile.TileContext(nc) as tc:
        from contextlib import ExitStack

        with ExitStack() as ctx:
            const = ctx.enter_context(tc.tile_pool(name="const", bufs=1))
            psip = ctx.enter_context(tc.tile_pool(name="psip", bufs=2))
            spool = ctx.enter_context(tc.tile_pool(name="spool", bufs=2))
            opool = ctx.enter_context(tc.tile_pool(name="opool", bufs=2))
            zpool = ctx.enter_context(tc.tile_pool(name="zpool", bufs=2))
            psum = ctx.enter_context(tc.tile_pool(name="psum", bufs=6, space="PSUM"))

            wt = const.tile([128, NWM * 128], dt.float16)
            nc.sync.dma_start(
                wt[:].rearrange("p (m k) -> p m k", m=NWM),
                wm.rearrange("m p k -> p m k"),
            )
            bt = const.tile([128, 2], dt.float32)
            nc.sync.dma_start(bt[:], bv[:])

            def W(i):
                return wt[:, i * 128 : (i + 1) * 128]

            for rep in range(reps):
              for sg in range(NSG):
                ptiles, s1tiles, s2tiles = [], [], []
                for v in range(4):
                    t = psip.tile([128, SG * XP], dt.float16, tag=f"psi{v}", name=f"psi_t{v}_{sg}")
                    nc.sync.dma_start(
                        t[:], psi[v, :, sg * SG * XP : (sg + 1) * SG * XP]
                    )
                    ptiles.append(t)
                    pv = t[:].rearrange("p (i x) -> p i x", i=SG)
                    t1 = spool.tile([128, SG * RES], dt.float16, tag=f"s1_{v}", name=f"s1_t{v}_{sg}")
                    t2 = spool.tile([128, SG * RES], dt.float16, tag=f"s2_{v}", name=f"s2_t{v}_{sg}")
                    v1 = t1[:].rearrange("p (i x) -> p i x", i=SG)
                    v2 = t2[:].rearrange("p (i x) -> p i x", i=SG)
                    nc.vector.tensor_add(v1, pv[:, :, 1 : 1 + RES], pv[:, :, 3 : 3 + RES])
                    nc.vector.tensor_add(v2, pv[:, :, 0:RES], pv[:, :, 4 : 4 + RES])
                    s1tiles.append(t1)
                    s2tiles.append(t2)
                otiles = [
                    opool.tile([128, SG * RES], dt.float16, tag=f"o{v}", name=f"o_t{v}_{sg}")
                    for v in range(4)
                ]

                def views(v, st):
                    # (center, s1, s2) rhs views of quarter v for step st
                    pt3 = ptiles[v][:].rearrange("p (i x) -> p i x", i=SG)[
                        :, st * S8 : (st + 1) * S8, :
                    ]
                    s13 = s1tiles[v][:].rearrange("p (i x) -> p i x", i=SG)[
                        :, st * S8 : (st + 1) * S8, :
                    ]
                    s23 = s2tiles[v][:].rearrange("p (i x) -> p i x", i=SG)[
                        :, st * S8 : (st + 1) * S8, :
                    ]
                    return (pt3[:, :, 2 : 2 + RES], s13, s23)

                for st in range(NST):
                    # stage 1: depthwise conv + lrelu (bias1 is always 0).
                    # One 1-bank accumulator per quarter, double-buffered.
                    z1 = []
                    for v in range(4):
                        cps = psum.tile(
                            [128, S8 * RES], dt.float32,
                            tag="cps", name=f"cps{sg}_{st}_{v}", bufs=2,
                        )
                        cps3 = cps[:].rearrange("p (i x) -> p i x", i=S8)
                        mm = []
                        own = views(v, st)
                        for c in range(3):  # lhsT c: x-class (0=s2,1=s1,2=center)
                            mm.append((W(c), own[2 - c]))
                        if v > 0:
                            below = views(v - 1, st)
                            for c in range(3):
                                mm.append((W(3 + c), below[2 - c]))
                        if v < 3:
                            above = views(v + 1, st)
                            for c in range(3):
                                mm.append((W(6 + c), above[2 - c]))
                        for j, (lhs, rhs) in enumerate(mm):
                            nc.tensor.matmul(
                                cps3, lhs, rhs,
                                start=(j == 0), stop=(j == len(mm) - 1),
                            )
                        z = zpool.tile(
                            [128, S8 * RES], dt.float16,
                            tag=f"z1_{v}", name=f"z1_{sg}_{st}_{v}",
                        )
                        nc.scalar.activation(z[:], cps[:], LR, alpha=0.01)
                        z1.append(z)
                    # stage 2: 8x8 channel GEMM + b2 + lrelu (one matmul/quarter)
                    z2 = []
                    for v in range(4):
                        gps = psum.tile(
                            [128, S8 * RES], dt.float32, tag="ps", name=f"g2ps{sg}_{st}_{v}",
                        )
                        nc.tensor.matmul(
                            gps[:], W(9), z1[v][:], start=True, stop=True,
                        )
                        z = zpool.tile([128, S8 * RES], dt.float16, tag=f"z2_{v}", name=f"z2_{sg}_{st}_{v}")
                        nc.scalar.activation(
                            z[:], gps[:], LR, bias=bt[:, 0:1], alpha=0.01
                        )
                        z2.append(z)
                    # stage 3: 8x8 channel GEMM + psi residual + b3 + tanh
                    for v in range(4):
                        gps = psum.tile(
                            [128, S8 * RES], dt.float32, tag="ps", name=f"g3ps{sg}_{st}_{v}",
                        )
                        pt3 = ptiles[v][:].rearrange("p (i x) -> p i x", i=SG)[
                            :, st * S8 : (st + 1) * S8, 2 : 2 + RES
                        ]
                        gps3 = gps[:].rearrange("p (i x) -> p i x", i=S8)
                        if resid_on_pe:
                            nc.tensor.matmul(gps3, W(11), pt3, start=True, stop=False)
                        nc.tensor.matmul(
                            gps[:], W(10), z2[v][:],
                            start=not resid_on_pe, stop=True,
                        )
                        if not resid_on_pe:
                            # psi residual on the vector engine
                            nc.vector.tensor_add(gps3, gps3, pt3)
                        nc.scalar.activation(
                            otiles[v][:, st * S8 * RES : (st + 1) * S8 * RES],
                            gps[:], TH, bias=bt[:, 1:2],
                        )
                for v in range(4):
                    nc.sync.dma_start(
                        out[v, :, sg * SG * RES : (sg + 1) * SG * RES],
                        otiles[v][:],
                    )

    nc.compile()
    return nc


def _host_pack(filter1, bias1, w2, b2, w3, b3):
    w = _totalistic(filter1.astype(np.float32))[:, 0]  # [8,5,5]
    wm = np.zeros((NWM, 128, 128), np.float32)
    # main banded conv matrices (same local structure for every quarter):
    # lhsT[ch*16+yi, ch*16+yo] = K[ch, (yi-yo)+2, c]
    for c in range(3):
        for ch in range(H):
            o = ch * YQ
            for yi in range(YQ):
                for yo in range(YQ):
                    d = yi - yo
                    if -2 <= d <= 2:
                        wm[c, o + yi, o + yo] = w[ch, d + 2, c]
            # spill up: src quarter u -> dst u+1; dy = ys - 16 - yd in {-2,-1}
            for ys, yd in ((14, 0), (15, 0), (15, 1)):
                wm[3 + c, o + ys, o + yd] = w[ch, (ys - 16 - yd) + 2, c]
            # spill down: src quarter u -> dst u-1; dy = ys + 16 - yd in {1,2}
            for ys, yd in ((0, 14), (0, 15), (1, 15)):
                wm[6 + c, o + ys, o + yd] = w[ch, (ys + 16 - yd) + 2, c]
    # channel GEMMs: lhsT[ci*16+y, co*16+y] = W[co, ci]
    i16 = np.eye(YQ, dtype=np.float32)
    for co in range(H):
        for ci in range(H):
            wm[9, ci * YQ : ci * YQ + YQ, co * YQ : co * YQ + YQ] = w2[co, ci] * i16
            wm[10, ci * YQ : ci * YQ + YQ, co * YQ : co * YQ + YQ] = w3[co, ci] * i16
    wm[11] = np.eye(128, dtype=np.float32)

    bvv = np.zeros((128, 2), np.float32)
    for ch in range(H):
        sl = slice(ch * YQ, ch * YQ + YQ)
        bvv[sl, 0] = b2[ch]
        bvv[sl, 1] = b3[ch]
    return wm.astype(F16), bvv


def _pack_psi(psi):
    """[BS,H,RES,RES] -> [NCORES, 4, 128, IPC*XP] fp16: tile axis = y-quarter,
    partition p = ch*16 + ylocal, free = (image, padded x)."""
    psip = np.zeros((BS, H, RES, XP), np.float32)
    psip[:, :, :, 2 : 2 + RES] = psi
    v = psip.reshape(NCORES, IPC, H, 4, YQ, XP)
    v = v.transpose(0, 3, 2, 4, 1, 5)  # [NCORES, qy, ch, ylocal, IPC, XP]
    return np.ascontiguousarray(v).reshape(NCORES, 4, 128, IPC * XP).astype(F16)


def _unpack_out(parts):
    """list of [4,128,IPC*RES] fp16 per core -> [BS,H,RES,RES] fp32"""
    v = np.stack([np.asarray(p).astype(np.float32) for p in parts])
    v = v.reshape(NCORES, 4, H, YQ, IPC, RES)
    v = v.transpose(0, 4, 2, 1, 3, 5)  # [NCORES, IPC, ch, qy, ylocal, RES]
    return np.ascontiguousarray(v).reshape(BS, H, RES, RES)


def kernel(psi, filter1, bias1, w2, b2, w3, b3):
    from concourse.bass_utils import run_bass_kernel_spmd

    psi = np.asarray(psi, dtype=np.float32)
    wm, bvv = _host_pack(
        np.asarray(filter1, np.float32),
        np.asarray(bias1, np.float32),
        np.asarray(w2, np.float32),
        np.asarray(b2, np.float32),
        np.asarray(w3, np.float32),
        np.asarray(b3, np.float32),
    )

    psit = _pack_psi(psi)

    if "nc" not in _CACHE:
        _CACHE["nc"] = _build_program()
    nc = _CACHE["nc"]

    in_maps = [{"psi": psit[c], "wm": wm, "bv": bvv} for c in range(NCORES)]
    res = run_bass_kernel_spmd(nc, in_maps, list(range(NCORES)))
    return _unpack_out([r["out"] for r in res.results])
